# revision 1
# baseline (speedup 1.0000x reference)
"""ConvLSTMCell Trainium2 kernel (8 NeuronCores, SPMD).

Problem (see reference): xi [4, 256, 16, 64, 64], W [256, 64, 3, 3], b [256]
  t=0:  gates from x0 directly, c0 = sig(i)*tanh(g), h0 = sig(o)*lrelu(c0)
  t>=1: tmp = conv3x3(h, W) + b + x_t;  c = sig(f)*c + sig(i)*tanh(g);
        h = sig(o)*lrelu(c)
Output: h stacked over t -> [4, 64, 16, 64, 64].

Sharding: 8 cores = (batch b, H-half). Each core computes a shrinking
redundant halo (region = its 32 rows + (15-t) extra rows toward the cut) so
there is NO inter-core communication. Bottom-half cores get their rows (and
W's ky axis) flipped host-side so all 8 cores run an identical program.

Per-core layout (channel-major):
  - out-channel permutation [i(0:64) f(64:128)] / [g(0:64) o(64:128)] so all
    elementwise ops are partition-band aligned (no cross-partition DVE ops).
  - conv = 7 matmul passes per 128-out-ch half per row-chunk:
      1 x K=128 identity (adds x_t into PSUM)
      3 x K=128 packed pair (partitions 64:128 = h padded, 0:64 = h shifted
        one col so one matmul covers taps (ky,kx=0)+(ky,kx=1))
      3 x K=64 singles (tap (ky,kx=2) read from the shifted copy)
  - gates: ACT sigmoid/tanh/lrelu with fused per-partition bias; c' = i*g+f*c
    via one DVE mul producing [ig; fc] + a stacked-identity matmul summing the
    bands on PE; h = o*l on GPSIMD. c state + h production all band-aligned.
  - fp32r matmuls (measured relerr ~1.5e-4 on HW).
"""
import numpy as np
from contextlib import ExitStack

import concourse.bacc as bacc
import concourse.tile as tile
from concourse import mybir
from concourse.bass_utils import run_bass_kernel_spmd

F32 = mybir.dt.float32
F32R = mybir.dt.float32r

B, CH4, T, HH, WW = 4, 256, 16, 64, 64
HID = 64
RG = 47            # region rows at t=0 (32 owned + 15 halo)
HP_R, WP = 48, 66  # padded h buffer rows/cols
CH_ROWS = 8        # rows per chunk
GROUP = 3          # chunks per psum group
NFLAT = RG * WW    # 3008

# weight blob columns
PK = [0, 256, 512]          # packed slabs ky=0,1,2  [128, 256]
SG = [768, 1024, 1280]      # single slabs ky=0,1,2  [64, 256] (upper rows 0)
IDC = 1536                  # identity 128           [128, 128]
IST = 1664                  # stacked identity       [128, 128]
WCOLS = 1792


def _emit_timestep_loop(nc, tc, pools, aps, repeats):
    consts, state, xp, hf, gp, op, lp, tp, psc = pools
    x_d, w_d, b_d, out_d = aps

    wb = consts.tile([128, WCOLS], F32R)
    bias = consts.tile([128, 2], F32)
    nc.sync.dma_start(out=wb, in_=w_d)
    nc.sync.dma_start(out=bias, in_=b_d)

    gc = state.tile([128, NFLAT], F32)       # [g ; c]
    hpadA = state.tile([128, HP_R * WP], F32R)
    hpadB = state.tile([128, HP_R * WP], F32R)
    hpads = [hpadA, hpadB]

    SIG = mybir.ActivationFunctionType.Sigmoid
    TANH = mybir.ActivationFunctionType.Tanh
    LRELU = mybir.ActivationFunctionType.Lrelu

    for _ in range(repeats):
        nc.vector.memset(hpadA.bitcast(F32), 0.0)
        nc.vector.memset(hpadB.bitcast(F32), 0.0)
        nc.vector.memset(gc[64:128, :], 0.0)

        for t in range(T):
            Ht = RG - t
            n_flat = Ht * WW
            nch = (Ht + CH_ROWS - 1) // CH_ROWS
            hp_w = hpads[t % 2]
            hp_r = hpads[(t + 1) % 2]
            hp3_w = hp_w.rearrange("p (r w) -> p r w", w=WP)
            hp3_r = hp_r.rearrange("p (r w) -> p r w", w=WP)

            xh = []
            for h in range(2):
                xt = xp.tile([128, NFLAT], F32R)
                x3 = xt.rearrange("p (r w) -> p r w", w=WW)
                nc.sync.dma_start(
                    out=x3[:, 0:Ht, :],
                    in_=x_d[t, 128 * h:128 * h + 128, 0:Ht, :])
                xh.append(xt)

            hfull = hf.tile([128, NFLAT], F32R)
            hf3 = hfull.rearrange("p (r w) -> p r w", w=WW)

            for g0 in range(0, nch, GROUP):
                chunks = list(range(g0, min(g0 + GROUP, nch)))
                ps = None
                if t > 0:
                    ps = [[psc.tile([128, CH_ROWS * WW], F32, tag="ps", name="ps")
                           for _ in chunks] for _ in range(2)]
                    for h in range(2):
                        hw = 128 * h
                        for ky in range(3):
                            for ci, c in enumerate(chunks):
                                rows = min(CH_ROWS, Ht - CH_ROWS * c)
                                n = rows * WW
                                r0 = CH_ROWS * c + ky
                                nc.tensor.matmul(
                                    ps[h][ci][:, 0:n],
                                    wb[:, PK[ky] + hw:PK[ky] + hw + 128],
                                    hp3_r[:, r0:r0 + rows, 0:64],
                                    start=(ky == 0), stop=False)
                        for ky in range(3):
                            for ci, c in enumerate(chunks):
                                rows = min(CH_ROWS, Ht - CH_ROWS * c)
                                n = rows * WW
                                r0 = CH_ROWS * c + ky
                                nc.tensor.matmul(
                                    ps[h][ci][:, 0:n],
                                    wb[0:64, SG[ky] + hw:SG[ky] + hw + 128],
                                    hp3_r[0:64, r0:r0 + rows, 1:65],
                                    start=False, stop=False)
                        for ci, c in enumerate(chunks):
                            rows = min(CH_ROWS, Ht - CH_ROWS * c)
                            n = rows * WW
                            nc.tensor.matmul(
                                ps[h][ci][:, 0:n],
                                wb[:, IDC:IDC + 128],
                                xh[h][:, 512 * c:512 * c + n],
                                start=False, stop=True)

                for ci, c in enumerate(chunks):
                    rows = min(CH_ROWS, Ht - CH_ROWS * c)
                    n = rows * WW
                    win = slice(512 * c, 512 * c + n)
                    if t > 0:
                        s0 = ps[0][ci][:, 0:n]
                        s1 = ps[1][ci][:, 0:n]
                        b0, b1 = bias[:, 0:1], bias[:, 1:2]
                    else:
                        s0 = xh[0][:, win]
                        s1 = xh[1][:, win]
                        b0, b1 = 0.0, 0.0

                    ifs = gp.tile([128, 512], F32)
                    nc.scalar.activation(ifs[:, 0:n], s0, SIG, bias=b0)
                    nc.scalar.activation(
                        gc[0:64, win], s1[0:64, :], TANH,
                        bias=(b1[0:64, :] if t > 0 else 0.0))
                    osb = op.tile([128, 512], F32)
                    nc.scalar.activation(
                        osb[64:128, 0:n], s1[64:128, :], SIG,
                        bias=(b1[64:128, :] if t > 0 else 0.0))

                    tmp = tp.tile([128, 512], F32R)
                    nc.vector.tensor_mul(tmp[:, 0:n], ifs[:, 0:n], gc[:, win])

                    if t > 0:
                        cps = ps[1][ci]
                    else:
                        cps = psc.tile([128, CH_ROWS * WW], F32, tag="ps", name="ps")
                    nc.tensor.matmul(cps[:, 0:n], wb[:, IST:IST + 128],
                                     tmp[:, 0:n], start=True, stop=True)

                    lsb = lp.tile([128, 512], F32)
                    nc.scalar.activation(lsb[64:128, 0:n], cps[64:128, 0:n],
                                         LRELU, alpha=0.01)
                    nc.vector.tensor_copy(gc[64:128, win], cps[64:128, 0:n])
                    nc.gpsimd.tensor_mul(hfull[64:128, win],
                                         osb[64:128, 0:n], lsb[64:128, 0:n])

                    r0 = CH_ROWS * c
                    if t < T - 1:
                        nc.sync.dma_start(
                            out=hp3_w[64:128, 1 + r0:1 + r0 + rows, 1:65],
                            in_=hf3[64:128, r0:r0 + rows, :])
                        nc.sync.dma_start(
                            out=hp3_w[0:64, 1 + r0:1 + r0 + rows, 0:64],
                            in_=hf3[64:128, r0:r0 + rows, :])
                    if r0 < 32:
                        srows = min(rows, 32 - r0)
                        nc.sync.dma_start(
                            out=out_d[:, t, r0:r0 + srows, :],
                            in_=hf3[64:128, r0:r0 + srows, :])


def build_nc(repeats=1):
    nc = bacc.Bacc("TRN2", target_bir_lowering=False, debug=False)
    x_d = nc.dram_tensor("x", [T, CH4, RG, WW], F32R,
                         kind="ExternalInput").ap()
    w_d = nc.dram_tensor("w", [128, WCOLS], F32R, kind="ExternalInput").ap()
    b_d = nc.dram_tensor("bias", [128, 2], F32, kind="ExternalInput").ap()
    out_d = nc.dram_tensor("out", [HID, T, 32, WW], F32R,
                           kind="ExternalOutput").ap()

    with tile.TileContext(nc) as tc, ExitStack() as ctx:
        consts = ctx.enter_context(tc.tile_pool(name="consts", bufs=1))
        state = ctx.enter_context(tc.tile_pool(name="state", bufs=1))
        xp = ctx.enter_context(tc.tile_pool(name="xp", bufs=6))
        hf = ctx.enter_context(tc.tile_pool(name="hf", bufs=2))
        gp = ctx.enter_context(tc.tile_pool(name="gp", bufs=4))
        op = ctx.enter_context(tc.tile_pool(name="op", bufs=3))
        lp = ctx.enter_context(tc.tile_pool(name="lp", bufs=3))
        tp = ctx.enter_context(tc.tile_pool(name="tp", bufs=4))
        psc = ctx.enter_context(
            tc.tile_pool(name="psc", bufs=8, space="PSUM"))
        _emit_timestep_loop(
            nc, tc, (consts, state, xp, hf, gp, op, lp, tp, psc),
            (x_d, w_d, b_d, out_d), repeats)
    nc.compile()
    return nc


def _prep_core_inputs(xi, W, b):
    """Host-side shard prep. Returns list of 8 in_maps."""
    # out-channel permutation: [i f g o]
    perm = np.concatenate([np.arange(0, 128), np.arange(192, 256),
                           np.arange(128, 192)])
    Wp = W[perm]                      # [256, 64, 3, 3]
    bp = b[perm]
    bias_blob = np.stack([bp[0:128], bp[128:256]], axis=1).astype(np.float32)
    bias_blob = np.ascontiguousarray(bias_blob)  # [128, 2]

    def wblob(Wv):
        wb = np.zeros((128, WCOLS), np.float32)
        for ky in range(3):
            wb[0:64, PK[ky]:PK[ky] + 256] = Wv[:, :, ky, 1].T
            wb[64:128, PK[ky]:PK[ky] + 256] = Wv[:, :, ky, 0].T
            wb[0:64, SG[ky]:SG[ky] + 256] = Wv[:, :, ky, 2].T
        wb[:, IDC:IDC + 128] = np.eye(128)
        ist = np.zeros((128, 128), np.float32)
        ist[0:64, 64:128] = np.eye(64)
        ist[64:128, 64:128] = np.eye(64)
        wb[:, IST:IST + 128] = ist
        return wb

    wb_top = wblob(Wp)
    wb_bot = wblob(Wp[:, :, ::-1, :])  # ky flipped for row-flipped cores

    in_maps = []
    for core in range(8):
        bb, half = divmod(core, 2)
        xs = xi[bb][perm]                      # [256, 16, 64, 64]
        if half == 0:
            xs = xs[:, :, 0:RG, :]
        else:
            xs = xs[:, :, ::-1, :][:, :, 0:RG, :]
        xs = np.ascontiguousarray(xs.transpose(1, 0, 2, 3))  # [16,256,47,64]
        in_maps.append({
            "x": xs.astype(np.float32),
            "w": (wb_top if half == 0 else wb_bot),
            "bias": bias_blob,
        })
    return in_maps


_NC_CACHE = {}


def kernel(xi, W, b):
    xi = np.asarray(xi, dtype=np.float32)
    W = np.asarray(W, dtype=np.float32)
    b = np.asarray(b, dtype=np.float32)
    if "nc" not in _NC_CACHE:
        _NC_CACHE["nc"] = build_nc(repeats=1)
    nc = _NC_CACHE["nc"]
    in_maps = _prep_core_inputs(xi, W, b)
    res = run_bass_kernel_spmd(nc, in_maps, list(range(8)), trace=False)
    out = np.empty((B, HID, T, HH, WW), np.float32)
    for core in range(8):
        bb, half = divmod(core, 2)
        o = res.results[core]["out"]          # [64, 16, 32, 64]
        o = np.moveaxis(o, 1, 1)              # [hid, T, 32, W]
        if half == 0:
            out[bb, :, :, 0:32, :] = o
        else:
            out[bb, :, :, 32:64, :] = o[:, :, ::-1, :]
    return out



# revision 12
# speedup vs baseline: 5.3822x; 5.3822x over previous
"""ConvLSTMCell Trainium2 kernel (8 NeuronCores, SPMD).

Problem (see reference): xi [4, 256, 16, 64, 64], W [256, 64, 3, 3], b [256]
  t=0:  gates from x0 directly, c0 = sig(i)*tanh(g), h0 = sig(o)*lrelu(c0)
  t>=1: tmp = conv3x3(h, W) + b + x_t;  c = sig(f)*c + sig(i)*tanh(g);
        h = sig(o)*lrelu(c)
Output: h stacked over t -> [4, 64, 16, 64, 64].

Sharding: 8 cores = (batch b, H-half). Each core computes a shrinking
redundant halo (region = its 32 rows + (15-t) extra rows toward the cut) so
there is NO inter-core communication. Bottom-half cores get their rows (and
W's ky axis) flipped host-side so all 8 cores run an identical program.

Per-core layout (channel-major, all PE operands bf16, accumulate fp32):
  - out-channel permutation [i(0:64) f(64:128)] / [g(0:64) o(64:128)] so all
    elementwise ops are partition-band aligned.
  - conv = 7 bf16 matmul passes per 128-out-ch half per row-chunk:
      3 x K=128 packed pair (partitions 64:128 = h padded 'P', 0:64 = h
        shifted one col 'S' so one matmul covers taps (ky,kx=0)+(ky,kx=1))
      3 x K=64 singles (tap (ky,kx=2) read from S at col+1)
      1 x K=128 identity (adds x_t into PSUM)
  - ACT uses ONLY Sigmoid+Tanh (one table set -> zero table reloads; the
    baseline's Lrelu lived in a different set and forced ~122 reloads).
  - gates (grouped over 2 chunks, [128,1024] PSUM tiles spanning 2 banks):
      sig [128,n] over [i;f], tanh [64,n] -> gc band0, sig [64,n] o.
  - DVE: prod = ifs*[g;c]; c' = band-add prod[0:64]+prod[64:128] written
    straight into the SBUF c-state (no stacked matmul, no PSUM->SBUF copy);
    h = lrelu(o*c') computed as one scalar_tensor_tensor max(0.01*p, p)
    after p = o*c' (valid because o = sigmoid > 0).
  - gpsimd writes h (bf16) directly into the next padded-h buffer 'P' band;
    DVE 4x bf16 copy produces the shifted 'S' band.
  - x streamed bf16; output stored bf16 and widened to fp32 on host.
"""
import numpy as np
from contextlib import ExitStack

import concourse.bacc as bacc
import concourse.tile as tile
from concourse import mybir
from concourse.bass_utils import run_bass_kernel_spmd

F32 = mybir.dt.float32
F32R = mybir.dt.float32r
BF16 = mybir.dt.bfloat16
NPBF16 = mybir.dt.np(BF16)

B, CH4, T, HH, WW = 4, 256, 16, 64, 64
HID = 64
RG = 47            # region rows at t=0 (32 owned + 15 halo)
HP_R, WP = 48, 66  # padded h buffer rows/cols
CH_ROWS = 8        # rows per chunk
GROUP = 2          # chunks per psum group (2 banks per [128,1024] tile)
NFLAT = RG * WW    # 3008

# weight blob columns (bf16)
PK = [0, 256, 512]          # packed slabs ky=0,1,2  [128, 256]
SG = [768, 1024, 1280]      # single slabs ky=0,1,2  [64, 256] (upper rows 0)
IDC = 1536                  # identity 128           [128, 128]
WCOLS = 1664

MULT = mybir.AluOpType.mult
MAX = mybir.AluOpType.max


def _emit_timestep_loop(nc, tc, pools, aps, repeats):
    consts, state, xp, gp, op, pp, lp, psc = pools
    x_d, w_d, b_d, ist_d, out_d = aps

    wb = consts.tile([128, WCOLS], BF16)
    bias = consts.tile([128, 2], F32)
    istw = consts.tile([128, 128], F32R)
    nc.sync.dma_start(out=wb, in_=w_d)
    nc.sync.dma_start(out=bias, in_=b_d)
    nc.sync.dma_start(out=istw, in_=ist_d)

    gc = state.tile([128, NFLAT], F32)       # [g ; c]
    hpadA = state.tile([128, HP_R * WP], BF16)
    hpadB = state.tile([128, HP_R * WP], BF16)
    hpads = [hpadA, hpadB]

    SIG = mybir.ActivationFunctionType.Sigmoid
    TANH = mybir.ActivationFunctionType.Tanh

    b0 = bias[:, 0:1]
    b1 = bias[:, 1:2]

    def emit_front(st):
        """Gate activations + i*g/f*c product (no PE instructions)."""
        (t, src0, src1, cg, wing, gn, coff, crows, rtot, r0g, hp3_w) = st
        if t > 0:
            bb0, bbg, bbo = b0, b1[0:64, :], b1[64:128, :]
        else:
            bb0, bbg, bbo = 0.0, 0.0, 0.0
        ifs = gp.tile([128, GROUP * 512], F32, name="ifs")
        osb = op.tile([128, GROUP * 512], F32, name="osb")
        nc.scalar.activation(gc[0:64, wing], src1[0:64, :], TANH, bias=bbg)
        nc.scalar.activation(osb[64:128, 0:gn], src1[64:128, :], SIG, bias=bbo)
        nc.scalar.activation(ifs[:, 0:gn], src0, SIG, bias=bb0)
        prod = pp.tile([128, GROUP * 512], F32R, name="prod")
        nc.vector.tensor_mul(prod[:, 0:gn], ifs[:, 0:gn], gc[:, wing])
        return (prod, osb)

    def emit_back(st, fr):
        """c' band-sum on PE + h production; emitted mid-next-group's convs
        so the PE reaches the stacked matmul after its DVE input is ready."""
        (t, src0, src1, cg, wing, gn, coff, crows, rtot, r0g, hp3_w) = st
        prod, osb = fr
        for ci in range(len(crows)):
            n = crows[ci] * WW
            nc.tensor.matmul(cg[:, coff[ci]:coff[ci] + n], istw,
                             prod[:, coff[ci]:coff[ci] + n],
                             start=True, stop=True)
        # p = o*c' straight from PSUM; h = lrelu(p) = max(0.01p, p) since o>0
        lt = lp.tile([128, GROUP * 512], F32, name="lt")
        nc.vector.tensor_mul(lt[64:128, 0:gn], osb[64:128, 0:gn],
                             cg[64:128, 0:gn])
        lt3 = lt.rearrange("p (r w) -> p r w", w=WW)
        nc.vector.scalar_tensor_tensor(
            hp3_w[64:128, 1 + r0g:1 + r0g + rtot, 1:65],
            lt3[64:128, 0:rtot, :], 0.01, lt3[64:128, 0:rtot, :],
            op0=MULT, op1=MAX)
        # shifted copy 'S' (band 0, col offset -1), bf16 4x mode
        nc.vector.tensor_copy(
            hp3_w[0:64, 1 + r0g:1 + r0g + rtot, 0:64],
            hp3_w[64:128, 1 + r0g:1 + r0g + rtot, 1:65])
        if r0g < 32:
            srows = min(rtot, 32 - r0g)
            nc.sync.dma_start(
                out=out_d[:, t, r0g:r0g + srows, :],
                in_=hp3_w[64:128, 1 + r0g:1 + r0g + srows, 1:65])
        # c state update (only needed by next timestep's prod)
        nc.vector.tensor_copy(gc[64:128, wing], cg[64:128, 0:gn])

    for _ in range(repeats):
        nc.vector.memset(hpadA, 0.0)
        nc.vector.memset(hpadB, 0.0)
        nc.vector.memset(gc[64:128, :], 0.0)

        xh_cache = {}

        def issue_x(t):
            if t >= T or t in xh_cache:
                return
            pair = []
            for h in range(2):
                xt = xp.tile([128, NFLAT], BF16, name="xt")
                x3 = xt.rearrange("p (r w) -> p r w", w=WW)
                nc.sync.dma_start(
                    out=x3[:, 0:RG - t, :],
                    in_=x_d[t, 128 * h:128 * h + 128, 0:RG - t, :])
                pair.append(xt)
            xh_cache[t] = pair

        issue_x(0)
        pending = None

        for t in range(T):
            Ht = RG - t
            nch = (Ht + CH_ROWS - 1) // CH_ROWS
            hp_w = hpads[t % 2]
            hp_r = hpads[(t + 1) % 2]
            hp3_w = hp_w.rearrange("p (r w) -> p r w", w=WP)
            hp3_r = hp_r.rearrange("p (r w) -> p r w", w=WP)

            issue_x(t + 1)   # prefetch next timestep's x
            xh = xh_cache.pop(t)

            for g0 in range(0, nch, GROUP):
                chunks = list(range(g0, min(g0 + GROUP, nch)))
                crows = [min(CH_ROWS, Ht - CH_ROWS * c) for c in chunks]
                coff = [0, crows[0] * WW]
                rtot = sum(crows)
                gn = rtot * WW
                r0g = CH_ROWS * g0
                base = r0g * WW      # flat col offset of the group
                wing = slice(base, base + gn)

                if t > 0:
                    s0g = psc.tile([128, GROUP * 512], F32, tag="ps", name="ps")
                    s1g = psc.tile([128, GROUP * 512], F32, tag="ps", name="ps")
                    for h, sg in ((1, s1g), (0, s0g)):
                        hw = 128 * h
                        for ky in range(3):
                            for ci, c in enumerate(chunks):
                                rows = crows[ci]
                                n = rows * WW
                                r0 = CH_ROWS * c + ky
                                nc.tensor.matmul(
                                    sg[:, coff[ci]:coff[ci] + n],
                                    wb[:, PK[ky] + hw:PK[ky] + hw + 128],
                                    hp3_r[:, r0:r0 + rows, 0:64],
                                    start=(ky == 0), stop=False)
                        for ky in range(3):
                            for ci, c in enumerate(chunks):
                                rows = crows[ci]
                                n = rows * WW
                                r0 = CH_ROWS * c + ky
                                nc.tensor.matmul(
                                    sg[:, coff[ci]:coff[ci] + n],
                                    wb[0:64, SG[ky] + hw:SG[ky] + hw + 128],
                                    hp3_r[0:64, r0:r0 + rows, 1:65],
                                    start=False, stop=False)
                        for ci, c in enumerate(chunks):
                            n = crows[ci] * WW
                            nc.tensor.matmul(
                                sg[:, coff[ci]:coff[ci] + n],
                                wb[:, IDC:IDC + 128],
                                xh[h][:, 512 * c:512 * c + n],
                                start=False, stop=True)
                        if h == 1 and pending is not None:
                            # back-half of the previous group lands here, so
                            # the PE reaches its stacked matmul ~14 matmuls
                            # after its DVE dependency started computing
                            emit_back(*pending)
                            pending = None
                    src0, src1 = s0g[:, 0:gn], s1g[:, 0:gn]
                    cg = s1g
                else:
                    if pending is not None:
                        emit_back(*pending)
                        pending = None
                    src0 = xh[0][:, base:base + gn]
                    src1 = xh[1][:, base:base + gn]
                    cg = psc.tile([128, GROUP * 512], F32, tag="ps", name="ps")

                st = (t, src0, src1, cg, wing, gn, coff, crows, rtot,
                      r0g, hp3_w)
                fr = emit_front(st)
                pending = (st, fr)

        if pending is not None:
            emit_back(*pending)
            pending = None


def build_nc(repeats=1):
    nc = bacc.Bacc("TRN2", target_bir_lowering=False, debug=False)
    x_d = nc.dram_tensor("x", [T, CH4, RG, WW], BF16,
                         kind="ExternalInput").ap()
    w_d = nc.dram_tensor("w", [128, WCOLS], BF16, kind="ExternalInput").ap()
    b_d = nc.dram_tensor("bias", [128, 2], F32, kind="ExternalInput").ap()
    ist_d = nc.dram_tensor("ist", [128, 128], F32R,
                           kind="ExternalInput").ap()
    out_d = nc.dram_tensor("out", [HID, T, 32, WW], BF16,
                           kind="ExternalOutput").ap()

    with tile.TileContext(nc) as tc, ExitStack() as ctx:
        consts = ctx.enter_context(tc.tile_pool(name="consts", bufs=1))
        state = ctx.enter_context(tc.tile_pool(name="state", bufs=1))
        xp = ctx.enter_context(tc.tile_pool(name="xp", bufs=6))
        gp = ctx.enter_context(tc.tile_pool(name="gp", bufs=3))
        op = ctx.enter_context(tc.tile_pool(name="op", bufs=3))
        pp = ctx.enter_context(tc.tile_pool(name="pp", bufs=3))
        lp = ctx.enter_context(tc.tile_pool(name="lp", bufs=3))
        psc = ctx.enter_context(
            tc.tile_pool(name="psc", bufs=4, space="PSUM"))
        _emit_timestep_loop(
            nc, tc, (consts, state, xp, gp, op, pp, lp, psc),
            (x_d, w_d, b_d, ist_d, out_d), repeats)
    nc.compile()
    return nc


def _prep_core_inputs(xi, W, b):
    """Host-side shard prep. Returns list of 8 in_maps."""
    # out-channel permutation: [i f g o]
    perm = np.concatenate([np.arange(0, 128), np.arange(192, 256),
                           np.arange(128, 192)])
    Wp = W[perm]                      # [256, 64, 3, 3]
    bp = b[perm]
    bias_blob = np.stack([bp[0:128], bp[128:256]], axis=1).astype(np.float32)
    bias_blob = np.ascontiguousarray(bias_blob)  # [128, 2]

    def wblob(Wv):
        wb = np.zeros((128, WCOLS), np.float32)
        for ky in range(3):
            wb[0:64, PK[ky]:PK[ky] + 256] = Wv[:, :, ky, 1].T
            wb[64:128, PK[ky]:PK[ky] + 256] = Wv[:, :, ky, 0].T
            wb[0:64, SG[ky]:SG[ky] + 256] = Wv[:, :, ky, 2].T
        wb[:, IDC:IDC + 128] = np.eye(128)
        return wb.astype(NPBF16)

    wb_top = wblob(Wp)
    wb_bot = wblob(Wp[:, :, ::-1, :])  # ky flipped for row-flipped cores
    ist = np.zeros((128, 128), np.float32)
    ist[0:64, 64:128] = np.eye(64)
    ist[64:128, 64:128] = np.eye(64)

    in_maps = []
    for core in range(8):
        bb, half = divmod(core, 2)
        xs = xi[bb][perm]                      # [256, 16, 64, 64]
        if half == 0:
            xs = xs[:, :, 0:RG, :]
        else:
            xs = xs[:, :, ::-1, :][:, :, 0:RG, :]
        xs = np.ascontiguousarray(xs.transpose(1, 0, 2, 3))  # [16,256,47,64]
        in_maps.append({
            "x": xs.astype(NPBF16),
            "w": (wb_top if half == 0 else wb_bot),
            "bias": bias_blob,
            "ist": ist,
        })
    return in_maps


_NC_CACHE = {}


def kernel(xi, W, b):
    xi = np.asarray(xi, dtype=np.float32)
    W = np.asarray(W, dtype=np.float32)
    b = np.asarray(b, dtype=np.float32)
    if "nc" not in _NC_CACHE:
        _NC_CACHE["nc"] = build_nc(repeats=1)
    nc = _NC_CACHE["nc"]
    in_maps = _prep_core_inputs(xi, W, b)
    res = run_bass_kernel_spmd(nc, in_maps, list(range(8)), trace=False)
    out = np.empty((B, HID, T, HH, WW), np.float32)
    for core in range(8):
        bb, half = divmod(core, 2)
        o = res.results[core]["out"].astype(np.float32)  # [64, 16, 32, 64]
        if half == 0:
            out[bb, :, :, 0:32, :] = o
        else:
            out[bb, :, :, 32:64, :] = o[:, :, ::-1, :]
    return out
